# revision 31
# baseline (speedup 1.0000x reference)
"""Trainium2 Bass kernel: sparse (sliding-window) attention block, v3.

Full module per reference:
  RMSNorm -> fused QKV (5120x2880) -> YaRN RoPE -> GQA sliding-window(128)
  causal attention with learned sink logit -> out projection (2880x4096).

Sharding: tensor-parallel over heads across 8 cores. Core c owns q-heads
[8c, 8c+8) and kv-head c. Each core emits a partial [1024, 2880] bf16
output; the host sums the partials (f64) and adds out_b.

v3 = v1 attention backend (q-on-partitions AV with ones-column
denominators, tiny reciprocals, PE transposes for the out-proj lhsT)
plus the v2 scheduling wins:
  - both activation tables (Ln, Exp) prefetched with dummy ops at t0; no
    Square activation anywhere (x^2 via vector bf16 muls) -> no table
    swaps, no startup stall.
  - half-granularity pipeline: kv+q projection and rope for tokens
    [0,512) are followed immediately by attention+out-proj of tiles 0-3
    while the second half's QKV matmuls still run.
  - sm_scale folded into the Exp activation scale -> q and k share one
    plain cos/sin table pair (half the table DMA/SBUF).
  - masks multiplicative {0,1} bf16 applied to the exp'd probabilities.
  - y written bf16 in [128, 960] chunks DMA'd from the scalar queue.
"""

import math
import sys

import numpy as np

try:
    import concourse.bass as bass
except ImportError:  # pragma: no cover
    sys.path.insert(0, "/opt/trn_rl_repo")
    import concourse.bass as bass

import concourse.bacc as bacc
import concourse.tile as tile
from concourse import mybir
from concourse.masks import make_identity
from concourse.bass_utils import run_bass_kernel_spmd

import ml_dtypes

BF16 = ml_dtypes.bfloat16

T = 1024
HIDDEN = 2880
HD = 64
NH = 64
NKV = 8
SW = 128
NCORES = 8
HPC = NH // NCORES          # q heads per core = 8
QKV_DIM = HD * (NH + 2 * NKV)
SM_SCALE = 1.0 / math.sqrt(HD)

P = 128
KT = (HIDDEN + P - 1) // P   # 23 k-tiles over hidden (zero-padded to 2944)
KPAD = KT * P
NT = 5                       # qkv n-tiles of 128 (4 q-tiles + 1 kv-tile)
MT = T // P                  # 8 token tiles
AW = HD + 1                  # AV width: 64 v dims + denominator column

dt = mybir.dt

_CACHE = {}


# ----------------------------------------------------------------------------
# host-side helpers
# ----------------------------------------------------------------------------

def _rope_cos_sin(num_tokens):
    base = 150000.0
    scaling = 32.0
    init_ctx = 4096.0
    ntk_alpha = 1.0
    ntk_beta = 32.0
    d_half = HD / 2
    freq = base ** (np.arange(0, HD, 2, dtype=np.float32) / HD)
    concentration = 0.1 * math.log(scaling) + 1.0
    low = d_half * math.log(init_ctx / (ntk_beta * 2 * math.pi)) / math.log(base)
    high = d_half * math.log(init_ctx / (ntk_alpha * 2 * math.pi)) / math.log(base)
    interpolation = 1.0 / (scaling * freq)
    extrapolation = 1.0 / freq
    ramp = (np.arange(int(d_half), dtype=np.float32) - low) / (high - low)
    m = 1.0 - np.clip(ramp, 0.0, 1.0)
    inv_freq = interpolation * (1.0 - m) + extrapolation * m
    t = np.arange(num_tokens, dtype=np.float32)
    freqs = t[:, None] * inv_freq[None, :]
    cos = (np.cos(freqs) * concentration).astype(np.float32)
    sin = (np.sin(freqs) * concentration).astype(np.float32)
    return cos, sin  # [T, 32]


def _host_tables():
    """Plain (unscaled) replicated rope tables with the swap sign folded
    into sin: rope(u)[p] = u[p]*cos[p] + u[p^32]*sin_alt[p]."""
    cos, sin = _rope_cos_sin(T)  # [1024, 32]
    sgn = np.repeat([-1.0, 1.0], 32)[:, None].astype(np.float32)
    sgn = np.tile(sgn, (2, 1))  # [128, 1]
    cos_t = np.tile(cos.T, (4, 1)).astype(np.float32)          # [128, 1024]
    sin_t = (np.tile(sin.T, (4, 1)) * sgn).astype(np.float32)  # [128, 1024]
    return cos_t, sin_t


def _host_masks01():
    j = np.arange(P)[:, None]   # kt row (partition)
    i = np.arange(P)[None, :]   # q col (free)
    mask_prev = (j > i).astype(np.float32)    # dist in [1,127]
    mask_self = (j <= i).astype(np.float32)   # dist in [0,127]
    return np.concatenate([mask_prev, mask_self], axis=1).astype(BF16)


def _prep_core_inputs(core, x, norm_scale, qkv_w, qkv_b, out_w, sinks):
    q_end = NH * HD
    k_end = q_end + NKV * HD

    qrows = np.arange(core * HPC * HD, (core + 1) * HPC * HD)
    krows = np.arange(q_end + core * HD, q_end + (core + 1) * HD)
    vrows = np.arange(k_end + core * HD, k_end + (core + 1) * HD)
    # kv n-tile: v in partitions 0:64, k in 64:128
    rows = np.concatenate([qrows, vrows, krows])  # [640]

    wshard = (qkv_w[rows, :] * norm_scale[None, :]).astype(np.float32)
    bshard = qkv_b[rows].astype(np.float32)  # [640]

    # lhsT tiles: wq[n, kp, kt*128 + nc] = wshard[n*128 + nc, kt*128 + kp]
    wq = np.zeros((NT, P, KPAD), dtype=BF16)
    for n in range(NT):
        blk = wshard[n * P:(n + 1) * P, :]  # [128 n, 2880 k]
        for ki in range(KT):
            k0 = ki * P
            ksz = min(P, HIDDEN - k0)
            wq[n, :ksz, ki * P:ki * P + P] = blk[:, k0:k0 + ksz].T.astype(BF16)

    cols = np.arange(core * HPC * HD, (core + 1) * HPC * HD)
    wo = out_w[:, cols].T.astype(np.float32)  # [512 hd, 2880 H]
    wout = wo.reshape(4, P, HIDDEN).astype(BF16)

    bqkv = bshard.reshape(NT, P).T.copy().astype(np.float32)  # [128, 5]

    cos_t, sin_t = _host_tables()  # [128, 1024] f32 each

    xt = np.zeros((KPAD, T), dtype=BF16)
    xt[:HIDDEN] = x.T.astype(BF16)

    esink = np.exp(sinks[core * HPC:(core + 1) * HPC].astype(np.float64))
    esink = np.broadcast_to(esink.astype(np.float32), (P, HPC)).copy()

    return {
        "xt": xt,                               # [2944, 1024] bf16
        "wq": wq,                               # [5, 128, 2944] bf16
        "wout": wout,                           # [4, 128, 2880] bf16
        "bqkv": bqkv,                           # [128, 5] f32
        "cos_t": cos_t, "sin_t": sin_t,         # [128, 1024] f32
        "mask": _host_masks01(),                # [128, 256] bf16
        "esink": esink,                         # [128, 8] f32
    }


# ----------------------------------------------------------------------------
# device kernel (Tile)
# ----------------------------------------------------------------------------

def build_nc():
    nc = bacc.Bacc("TRN2", target_bir_lowering=False, debug=False)

    xt_d = nc.dram_tensor("xt", [KPAD, T], dt.bfloat16, kind="ExternalInput").ap()
    wq_d = nc.dram_tensor("wq", [NT, P, KPAD], dt.bfloat16, kind="ExternalInput").ap()
    wout_d = nc.dram_tensor("wout", [4, P, HIDDEN], dt.bfloat16, kind="ExternalInput").ap()
    bqkv_d = nc.dram_tensor("bqkv", [P, NT], dt.float32, kind="ExternalInput").ap()
    cos_d = nc.dram_tensor("cos_t", [P, T], dt.float32, kind="ExternalInput").ap()
    sin_d = nc.dram_tensor("sin_t", [P, T], dt.float32, kind="ExternalInput").ap()
    mask_d = nc.dram_tensor("mask", [P, 2 * P], dt.bfloat16, kind="ExternalInput").ap()
    esink_d = nc.dram_tensor("esink", [P, HPC], dt.float32, kind="ExternalInput").ap()
    y_d = nc.dram_tensor("y", [T, HIDDEN], dt.bfloat16, kind="ExternalOutput").ap()

    YC = 480

    def bcast_mid(ap2d, n):
        """[P, F] -> [P, n, F] with a 0-step middle dim (free broadcast)."""
        return bass.AP(tensor=ap2d.tensor, offset=ap2d.offset,
                       ap=[ap2d.ap[0], [0, n]] + list(ap2d.ap[1:]))

    with tile.TileContext(nc) as tc:
        with (
            tc.tile_pool(name="const", bufs=1) as const,
            tc.tile_pool(name="res", bufs=1) as res,
            tc.tile_pool(name="qkvp", bufs=2) as qkvp,
            tc.tile_pool(name="xsqp", bufs=2) as xsqp,
            tc.tile_pool(name="ropep", bufs=2) as ropep,
            tc.tile_pool(name="ptp", bufs=3) as ptp,
            tc.tile_pool(name="smallp", bufs=3) as smallp,
            tc.tile_pool(name="anp", bufs=3) as anp,
            tc.tile_pool(name="attp", bufs=3) as attp,
            tc.tile_pool(name="ysbp", bufs=3) as ysbp,
            tc.tile_pool(name="pA", bufs=2, space="PSUM") as pA,
            tc.tile_pool(name="pY", bufs=3, space="PSUM") as pY,
            tc.tile_pool(name="pS", bufs=3, space="PSUM") as pS,
        ):
            # ---- constants ----
            zbias = const.tile([P, 1], dt.float32, tag="zbias", name="zbias")
            nc.vector.memset(zbias, 0.0)
            eps_t = const.tile([1, 1], dt.float32, tag="eps", name="eps_t")
            nc.vector.memset(eps_t, 1e-5)
            ones = const.tile([P, 1], dt.bfloat16, tag="ones", name="ones")
            nc.vector.memset(ones, 1.0)
            identb = const.tile([P, P], dt.bfloat16, tag="identb", name="identb")
            make_identity(nc, identb)
            # prefetch both activation tables (Ln, Exp) with dummy ops
            dmy = const.tile([1, 2], dt.float32, tag="dmy", name="dmy")
            nc.scalar.activation(dmy[:, 0:1], eps_t,
                                 mybir.ActivationFunctionType.Ln,
                                 bias=eps_t)
            nc.scalar.activation(dmy[:, 1:2], eps_t,
                                 mybir.ActivationFunctionType.Exp,
                                 bias=zbias[0:1, :])

            # ---- DMA issue (sync queue; y outputs go on the scalar queue) ----
            wq_sb = [res.tile([P, KPAD], dt.bfloat16, tag=f"wq{n}", name=f"wq{n}")
                     for n in range(NT)]
            xt_sb = res.tile([P, KT, T], dt.bfloat16, tag="xt", name="xt")
            wout_sb = [res.tile([P, HIDDEN], dt.bfloat16, tag=f"wout{kk}",
                                name=f"wout{kk}") for kk in range(4)]
            cos_sb = const.tile([P, T], dt.float32, tag="cos", name="cos_sb")
            sin_sb = const.tile([P, T], dt.float32, tag="sin", name="sin_sb")
            mask_sb = const.tile([P, 2 * P], dt.bfloat16, tag="mask", name="mask_sb")
            esink_sb = const.tile([P, HPC], dt.float32, tag="esink", name="esink_sb")
            bqkv_sb = const.tile([P, NT], dt.float32, tag="bqkv", name="bqkv_sb")

            HK = KPAD // 2
            HO = HIDDEN // 2
            HT = T // 2

            def dma(out, in_):
                nc.sync.dma_start(out=out, in_=in_)

            def dma2(out, in_):
                nc.scalar.dma_start(out=out, in_=in_)

            # xt first (rsq path is the long pole); first tile split for an
            # early pipeline start; weights interleaved in need order
            dma(xt_sb[:, 0, 0:512], xt_d[0:P, 0:512])
            dma2(xt_sb[:, 0, 512:], xt_d[0:P, 512:])
            dma(wq_sb[4][:, :HK], wq_d[4, :, :HK])
            for ki in range(1, 5):
                (dma if ki % 2 else dma2)(xt_sb[:, ki, :],
                                          xt_d[ki * P:(ki + 1) * P, :])
            dma(wq_sb[4][:, HK:], wq_d[4, :, HK:])
            for ki in range(5, 9):
                (dma if ki % 2 else dma2)(xt_sb[:, ki, :],
                                          xt_d[ki * P:(ki + 1) * P, :])
            dma(wq_sb[0][:, :HK], wq_d[0, :, :HK])
            for ki in range(9, 13):
                (dma if ki % 2 else dma2)(xt_sb[:, ki, :],
                                          xt_d[ki * P:(ki + 1) * P, :])
            dma(wq_sb[0][:, HK:], wq_d[0, :, HK:])
            for ki in range(13, 18):
                (dma if ki % 2 else dma2)(xt_sb[:, ki, :],
                                          xt_d[ki * P:(ki + 1) * P, :])
            dma(cos_sb[:, :HT], cos_d[:, :HT])
            dma2(sin_sb[:, :HT], sin_d[:, :HT])
            for ki in range(18, KT):
                (dma if ki % 2 else dma2)(xt_sb[:, ki, :],
                                          xt_d[ki * P:(ki + 1) * P, :])
            dma(cos_sb[:, HT:], cos_d[:, HT:])
            dma2(sin_sb[:, HT:], sin_d[:, HT:])
            dma(mask_sb, mask_d)
            dma(esink_sb, esink_d)
            dma(bqkv_sb, bqkv_d)
            dma(wq_sb[1][:, :HK], wq_d[1, :, :HK])
            dma(wq_sb[1][:, HK:], wq_d[1, :, HK:])
            dma(wout_sb[0][:, :HO], wout_d[0, :, :HO])
            dma(wout_sb[0][:, HO:], wout_d[0, :, HO:])
            dma(wq_sb[2][:, :HK], wq_d[2, :, :HK])
            dma(wq_sb[2][:, HK:], wq_d[2, :, HK:])
            for kk in range(1, 4):
                dma(wout_sb[kk][:, :HO], wout_d[kk, :, :HO])
                dma(wout_sb[kk][:, HO:], wout_d[kk, :, HO:])
            dma(wq_sb[3][:, :HK], wq_d[3, :, :HK])
            dma(wq_sb[3][:, HK:], wq_d[3, :, HK:])

            # ---- sum of squares: vector bf16 squares + PE ones-reduction ----
            psum_ssq = [pY.tile([1, 512], dt.float32, tag="py", name=f"ssq{h}")
                        for h in range(2)]  # holds 2 of pY's 3 bufs until rsq
            for ki in range(KT):
                xsq = xsqp.tile([P, T], dt.bfloat16, tag="xsq", name="xsq")
                eng = nc.gpsimd if ki % 3 == 2 else nc.vector
                eng.tensor_mul(xsq, xt_sb[:, ki, :], xt_sb[:, ki, :])
                for half in range(2):
                    nc.tensor.matmul(
                        psum_ssq[half],
                        ones,
                        xsq[:, half * 512:half * 512 + 512],
                        start=(ki == 0), stop=(ki == KT - 1),
                    )

            # rsq_b = exp(-0.5*ln(ssq/H + eps)) broadcast to 128 partitions
            lnm = res.tile([1, T], dt.float32, tag="lnm", name="lnm")
            for half in range(2):
                nc.scalar.activation(lnm[:, half * 512:half * 512 + 512],
                                     psum_ssq[half],
                                     mybir.ActivationFunctionType.Ln,
                                     bias=eps_t, scale=1.0 / HIDDEN)
            rinv = res.tile([1, T], dt.float32, tag="rinv", name="rinv")
            nc.scalar.activation(rinv, lnm, mybir.ActivationFunctionType.Exp,
                                 bias=zbias[0:1, :], scale=-0.5)
            rsq_b = res.tile([P, T], dt.float32, tag="rsq", name="rsq_b")
            nc.gpsimd.partition_broadcast(rsq_b, rinv)

            # ---- qkv projection (scaled+biased, bf16) ----
            def qkv_tile(n, half, dst):
                """dst <- (W x)*rsq + b for columns [half*512, ...+512)."""
                c0 = half * 512
                pq = pA.tile([P, 512], dt.float32, tag="pb", name="pb")
                for ki in range(KT):
                    nc.tensor.matmul(
                        pq,
                        wq_sb[n][:, ki * P:ki * P + P],
                        xt_sb[:, ki, c0:c0 + 512],
                        start=(ki == 0), stop=(ki == KT - 1),
                    )
                nc.vector.tensor_mul(dst, pq, rsq_b[:, c0:c0 + 512])
                nc.vector.tensor_scalar_add(dst, dst, bqkv_sb[:, n:n + 1])

            qra = res.tile([64, HPC, T], dt.bfloat16, tag="qra", name="qra")
            krope = res.tile([64, T], dt.bfloat16, tag="krope", name="krope")
            # all 8 token-major v tiles in one buffer; ones column prefilled
            vtok = res.tile([P, MT, AW], dt.bfloat16, tag="vtok", name="vtok")
            nc.vector.memset(vtok[:, :, HD:HD + 1], 1.0)

            qkvT4 = res.tile([P, T], dt.bfloat16, tag="qkvT4", name="qkvT4")

            def kv_phase(half):
                c0 = half * 512
                qkv_tile(4, half, qkvT4[:, c0:c0 + 512])
                for j in range(4):
                    b = half * 4 + j
                    pv = pS.tile([P, HD], dt.bfloat16, tag="sc", name="pv")
                    nc.tensor.transpose(pv, qkvT4[0:64, b * P:(b + 1) * P],
                                        identb[:64, :64])
                    nc.vector.tensor_copy(vtok[:, b, 0:HD], pv)
                # k rope (rows 64:128) at half width
                kswp = ropep.tile([P, 512], dt.bfloat16, tag="hswp", name="kswp")
                for a in (64, 96):
                    nc.scalar.copy(kswp[a:a + 32, :],
                                   qkvT4[a ^ 32:(a ^ 32) + 32, c0:c0 + 512])
                ktc = ropep.tile([P, 512], dt.bfloat16, tag="htc", name="ktc")
                nc.vector.tensor_mul(ktc[64:128, :], qkvT4[64:128, c0:c0 + 512],
                                     cos_sb[64:128, c0:c0 + 512])
                kts = ropep.tile([P, 512], dt.bfloat16, tag="hts", name="kts")
                nc.vector.tensor_mul(kts[64:128, :], kswp[64:128, :],
                                     sin_sb[64:128, c0:c0 + 512])
                nc.vector.tensor_add(krope[:, c0:c0 + 512], ktc[64:128, :],
                                     kts[64:128, :])

            def q_phase(half):
                c0 = half * 512
                for n in range(4):
                    qkvt = qkvp.tile([P, 512], dt.bfloat16, tag="qkvT",
                                     name=f"qkvt{n}")
                    qkv_tile(n, half, qkvt)
                    # rope at half width on the fly (swaps on the scalar queue)
                    swp = ropep.tile([P, 512], dt.bfloat16, tag="hswp", name="hswp")
                    for a in range(0, P, 32):
                        nc.scalar.copy(swp[a:a + 32, :],
                                       qkvt[a ^ 32:(a ^ 32) + 32, :])
                    tc_ = ropep.tile([P, 512], dt.bfloat16, tag="htc", name="htc")
                    nc.vector.tensor_mul(tc_, qkvt, cos_sb[:, c0:c0 + 512])
                    ts_ = ropep.tile([P, 512], dt.bfloat16, tag="hts", name="hts")
                    nc.vector.tensor_mul(ts_, swp, sin_sb[:, c0:c0 + 512])
                    for i in range(2):
                        b0 = 64 * i
                        nc.vector.tensor_add(qra[:, 2 * n + i, c0:c0 + 512],
                                             tc_[b0:b0 + 64, :],
                                             ts_[b0:b0 + 64, :])

            # ---- attention + out-projection for one token tile ----
            def attention(b):
                pt = ptp.tile([P, 2, HPC, P], dt.bfloat16, tag="pt", name="pt")
                kts = [(0, b - 1), (1, b)] if b > 0 else [(1, b)]
                for s, kt in kts:
                    for g in range(2):
                        ps = pS.tile([P, 4, P], dt.float32, tag="sc", name="sc")
                        nc.tensor.matmul(
                            ps,
                            krope[:, kt * P:(kt + 1) * P],
                            qra[:, 4 * g:4 * g + 4, b * P:(b + 1) * P],
                            start=True, stop=True)
                        nc.scalar.activation(pt[:, s, 4 * g:4 * g + 4, :], ps,
                                             mybir.ActivationFunctionType.Exp,
                                             bias=zbias, scale=SM_SCALE)
                    m0 = 0 if s == 0 else P
                    nc.vector.tensor_mul(pt[:, s], pt[:, s],
                                         bcast_mid(mask_sb[:, m0:m0 + P], HPC))

                # AV with ones column -> denominators in column 64
                rec8 = smallp.tile([P, HPC], dt.float32, tag="rec8", name="rec8")
                an = anp.tile([P, HPC, HD], dt.bfloat16, tag="an", name="an")
                for g in range(2):
                    pg = pS.tile([P, 4, AW], dt.float32, tag="sc", name="pg")
                    for j in range(4):
                        h = 4 * g + j
                        for idx, (s, kt) in enumerate(kts):
                            nc.tensor.matmul(pg[:, j, :], pt[:, s, h, :],
                                             vtok[:, kt, :],
                                             start=(idx == 0),
                                             stop=(idx == len(kts) - 1))
                    g0 = 4 * g
                    nc.vector.tensor_add(rec8[:, g0:g0 + 4],
                                         pg[:, :, HD:HD + 1],
                                         esink_sb[:, g0:g0 + 4])
                    nc.vector.reciprocal(rec8[:, g0:g0 + 4], rec8[:, g0:g0 + 4])
                    rec3 = bass.AP(tensor=rec8.tensor,
                                   offset=rec8[:, g0:g0 + 4].offset,
                                   ap=[rec8.ap[0], [1, 4], [0, HD]])
                    nc.vector.tensor_mul(an[:, g0:g0 + 4, :], pg[:, :, 0:HD],
                                         rec3)

                # transpose to out-proj lhsT layout [128 hd, 128 tok]
                att = attp.tile([P, 4, P], dt.bfloat16, tag="att", name="att")
                a2 = an.rearrange("p a b -> p (a b)")
                for kk in range(4):
                    pat = pS.tile([P, P], dt.bfloat16, tag="sc", name="pat")
                    nc.tensor.transpose(pat, a2[:, kk * P:(kk + 1) * P], identb)
                    if kk % 2 == 0:
                        nc.vector.tensor_copy(att[:, kk, :], pat)
                    else:
                        nc.scalar.copy(att[:, kk, :], pat)

                # out projection, bf16 out; y DMAs ride the scalar queue
                # (the last tile is drained in 480-chunks on both queues to
                # shorten the kernel tail)
                for hy in range(3):
                    ysb = ysbp.tile([P, 2, YC], dt.bfloat16, tag="ysb", name="ysb")
                    for j in range(2):
                        ch = 2 * hy + j
                        o0 = ch * YC
                        pyt = pY.tile([P, 512], dt.float32, tag="py", name="py")
                        for kk in range(4):
                            nc.tensor.matmul(pyt[:, 0:YC], att[:, kk, :],
                                             wout_sb[kk][:, o0:o0 + YC],
                                             start=(kk == 0), stop=(kk == 3))
                        if ch % 2 == 0:
                            nc.vector.tensor_copy(ysb[:, j, :], pyt[:, 0:YC])
                        else:
                            nc.scalar.copy(ysb[:, j, :], pyt[:, 0:YC])
                        if b == MT - 1:
                            eng = nc.sync if ch % 2 == 0 else nc.scalar
                            eng.dma_start(
                                out=y_d[b * P:(b + 1) * P, o0:o0 + YC],
                                in_=ysb[:, j, :])
                    if b < MT - 1:
                        nc.scalar.dma_start(
                            out=y_d[b * P:(b + 1) * P,
                                    hy * 2 * YC:(hy + 1) * 2 * YC],
                            in_=ysb)

            for half in range(2):
                kv_phase(half)
                q_phase(half)
                for j in range(4):
                    attention(half * 4 + j)

    nc.compile()
    return nc


# ----------------------------------------------------------------------------
# public entry
# ----------------------------------------------------------------------------

LAST_RESULTS = None


def kernel(x, norm_scale, qkv_w, qkv_b, out_w, out_b, sinks):
    global LAST_RESULTS
    x = np.asarray(x, dtype=np.float32)
    norm_scale = np.asarray(norm_scale, dtype=np.float32)
    qkv_w = np.asarray(qkv_w, dtype=np.float32)
    qkv_b = np.asarray(qkv_b, dtype=np.float32)
    out_w = np.asarray(out_w, dtype=np.float32)
    out_b = np.asarray(out_b, dtype=np.float32)
    sinks = np.asarray(sinks, dtype=np.float32)

    if "nc" not in _CACHE:
        _CACHE["nc"] = build_nc()
    nc = _CACHE["nc"]

    in_maps = [
        _prep_core_inputs(c, x, norm_scale, qkv_w, qkv_b, out_w, sinks)
        for c in range(NCORES)
    ]
    import os
    tmpdir = os.environ.get("BASS_TMPDIR") or None
    res = run_bass_kernel_spmd(nc, in_maps, core_ids=list(range(NCORES)),
                               tmpdir=tmpdir)
    LAST_RESULTS = res
    y = np.zeros((T, HIDDEN), dtype=np.float64)
    for c in range(NCORES):
        y += res.results[c]["y"].astype(np.float64)
    y += out_b.astype(np.float64)[None, :]
    return y.astype(np.float32)


# revision 35
# speedup vs baseline: 1.1286x; 1.1286x over previous
"""Trainium2 Bass kernel: sparse (sliding-window) attention block, v3.

Full module per reference:
  RMSNorm -> fused QKV (5120x2880) -> YaRN RoPE -> GQA sliding-window(128)
  causal attention with learned sink logit -> out projection (2880x4096).

Sharding: tensor-parallel over heads across 8 cores. Core c owns q-heads
[8c, 8c+8) and kv-head c. Each core emits a partial [1024, 2880] bf16
output; the host sums the partials (f64) and adds out_b.

v3 = v1 attention backend (q-on-partitions AV with ones-column
denominators, tiny reciprocals, PE transposes for the out-proj lhsT)
plus the v2 scheduling wins:
  - both activation tables (Ln, Exp) prefetched with dummy ops at t0; no
    Square activation anywhere (x^2 via vector bf16 muls) -> no table
    swaps, no startup stall.
  - half-granularity pipeline: kv+q projection and rope for tokens
    [0,512) are followed immediately by attention+out-proj of tiles 0-3
    while the second half's QKV matmuls still run.
  - sm_scale folded into the Exp activation scale -> q and k share one
    plain cos/sin table pair (half the table DMA/SBUF).
  - masks multiplicative {0,1} bf16 applied to the exp'd probabilities.
  - y written bf16 in [128, 960] chunks DMA'd from the scalar queue.
"""

import math
import sys

import numpy as np

try:
    import concourse.bass as bass
except ImportError:  # pragma: no cover
    sys.path.insert(0, "/opt/trn_rl_repo")
    import concourse.bass as bass

import concourse.bacc as bacc
import concourse.tile as tile
from concourse import mybir
from concourse.masks import make_identity
from concourse.bass_utils import run_bass_kernel_spmd

import ml_dtypes

BF16 = ml_dtypes.bfloat16

T = 1024
HIDDEN = 2880
HD = 64
NH = 64
NKV = 8
SW = 128
NCORES = 8
HPC = NH // NCORES          # q heads per core = 8
QKV_DIM = HD * (NH + 2 * NKV)
SM_SCALE = 1.0 / math.sqrt(HD)

P = 128
KT = (HIDDEN + P - 1) // P   # 23 k-tiles over hidden (zero-padded to 2944)
KPAD = KT * P
NT = 5                       # qkv n-tiles of 128 (4 q-tiles + 1 kv-tile)
MT = T // P                  # 8 token tiles
AW = HD + 1                  # AV width: 64 v dims + denominator column

dt = mybir.dt

_CACHE = {}


# ----------------------------------------------------------------------------
# host-side helpers
# ----------------------------------------------------------------------------

def _rope_cos_sin(num_tokens):
    base = 150000.0
    scaling = 32.0
    init_ctx = 4096.0
    ntk_alpha = 1.0
    ntk_beta = 32.0
    d_half = HD / 2
    freq = base ** (np.arange(0, HD, 2, dtype=np.float32) / HD)
    concentration = 0.1 * math.log(scaling) + 1.0
    low = d_half * math.log(init_ctx / (ntk_beta * 2 * math.pi)) / math.log(base)
    high = d_half * math.log(init_ctx / (ntk_alpha * 2 * math.pi)) / math.log(base)
    interpolation = 1.0 / (scaling * freq)
    extrapolation = 1.0 / freq
    ramp = (np.arange(int(d_half), dtype=np.float32) - low) / (high - low)
    m = 1.0 - np.clip(ramp, 0.0, 1.0)
    inv_freq = interpolation * (1.0 - m) + extrapolation * m
    t = np.arange(num_tokens, dtype=np.float32)
    freqs = t[:, None] * inv_freq[None, :]
    cos = (np.cos(freqs) * concentration).astype(np.float32)
    sin = (np.sin(freqs) * concentration).astype(np.float32)
    return cos, sin  # [T, 32]


def _host_tables():
    """Plain (unscaled) replicated rope tables with the swap sign folded
    into sin: rope(u)[p] = u[p]*cos[p] + u[p^32]*sin_alt[p]."""
    cos, sin = _rope_cos_sin(T)  # [1024, 32]
    sgn = np.repeat([-1.0, 1.0], 32)[:, None].astype(np.float32)
    sgn = np.tile(sgn, (2, 1))  # [128, 1]
    cos_t = np.tile(cos.T, (4, 1)).astype(np.float32)          # [128, 1024]
    sin_t = (np.tile(sin.T, (4, 1)) * sgn).astype(np.float32)  # [128, 1024]
    return cos_t, sin_t


def _host_masks01():
    j = np.arange(P)[:, None]   # kt row (partition)
    i = np.arange(P)[None, :]   # q col (free)
    mask_prev = (j > i).astype(np.float32)    # dist in [1,127]
    mask_self = (j <= i).astype(np.float32)   # dist in [0,127]
    return np.concatenate([mask_prev, mask_self], axis=1).astype(BF16)


def _prep_core_inputs(core, x, norm_scale, qkv_w, qkv_b, out_w, sinks):
    q_end = NH * HD
    k_end = q_end + NKV * HD

    qrows = np.arange(core * HPC * HD, (core + 1) * HPC * HD)
    krows = np.arange(q_end + core * HD, q_end + (core + 1) * HD)
    vrows = np.arange(k_end + core * HD, k_end + (core + 1) * HD)
    # kv n-tile: v in partitions 0:64, k in 64:128
    rows = np.concatenate([qrows, vrows, krows])  # [640]

    wshard = (qkv_w[rows, :] * norm_scale[None, :]).astype(np.float32)
    bshard = qkv_b[rows].astype(np.float32)  # [640]

    # lhsT tiles: wq[n, kp, kt*128 + nc] = wshard[n*128 + nc, kt*128 + kp]
    wq = np.zeros((NT, P, KPAD), dtype=BF16)
    for n in range(NT):
        blk = wshard[n * P:(n + 1) * P, :]  # [128 n, 2880 k]
        for ki in range(KT):
            k0 = ki * P
            ksz = min(P, HIDDEN - k0)
            wq[n, :ksz, ki * P:ki * P + P] = blk[:, k0:k0 + ksz].T.astype(BF16)

    cols = np.arange(core * HPC * HD, (core + 1) * HPC * HD)
    wo = out_w[:, cols].T.astype(np.float32)  # [512 hd, 2880 H]
    wout = wo.reshape(4, P, HIDDEN).astype(BF16)

    bqkv = bshard.reshape(NT, P).T.copy().astype(np.float32)  # [128, 5]

    cos_t, sin_t = _host_tables()  # [128, 1024] f32 each

    xt = np.zeros((KPAD, T), dtype=BF16)
    xt[:HIDDEN] = x.T.astype(BF16)

    esink = np.exp(sinks[core * HPC:(core + 1) * HPC].astype(np.float64))
    esink = np.broadcast_to(esink.astype(np.float32), (P, HPC)).copy()

    return {
        "xt": xt,                               # [2944, 1024] bf16
        "wq": wq,                               # [5, 128, 2944] bf16
        "wout": wout,                           # [4, 128, 2880] bf16
        "bqkv": bqkv,                           # [128, 5] f32
        "cos_t": cos_t, "sin_t": sin_t,         # [128, 1024] f32
        "mask": _host_masks01(),                # [128, 256] bf16
        "esink": esink,                         # [128, 8] f32
    }


# ----------------------------------------------------------------------------
# device kernel (Tile)
# ----------------------------------------------------------------------------

def build_nc():
    nc = bacc.Bacc("TRN2", target_bir_lowering=False, debug=False)

    xt_d = nc.dram_tensor("xt", [KPAD, T], dt.bfloat16, kind="ExternalInput").ap()
    wq_d = nc.dram_tensor("wq", [NT, P, KPAD], dt.bfloat16, kind="ExternalInput").ap()
    wout_d = nc.dram_tensor("wout", [4, P, HIDDEN], dt.bfloat16, kind="ExternalInput").ap()
    bqkv_d = nc.dram_tensor("bqkv", [P, NT], dt.float32, kind="ExternalInput").ap()
    cos_d = nc.dram_tensor("cos_t", [P, T], dt.float32, kind="ExternalInput").ap()
    sin_d = nc.dram_tensor("sin_t", [P, T], dt.float32, kind="ExternalInput").ap()
    mask_d = nc.dram_tensor("mask", [P, 2 * P], dt.bfloat16, kind="ExternalInput").ap()
    esink_d = nc.dram_tensor("esink", [P, HPC], dt.float32, kind="ExternalInput").ap()
    y_d = nc.dram_tensor("y", [T, HIDDEN], dt.bfloat16, kind="ExternalOutput").ap()

    YC = 480

    def bcast_mid(ap2d, n):
        """[P, F] -> [P, n, F] with a 0-step middle dim (free broadcast)."""
        return bass.AP(tensor=ap2d.tensor, offset=ap2d.offset,
                       ap=[ap2d.ap[0], [0, n]] + list(ap2d.ap[1:]))

    with tile.TileContext(nc) as tc:
        with (
            tc.tile_pool(name="const", bufs=1) as const,
            tc.tile_pool(name="res", bufs=1) as res,
            tc.tile_pool(name="qkvp", bufs=2) as qkvp,
            tc.tile_pool(name="xsqp", bufs=2) as xsqp,
            tc.tile_pool(name="ropep", bufs=2) as ropep,
            tc.tile_pool(name="ptp", bufs=3) as ptp,
            tc.tile_pool(name="smallp", bufs=3) as smallp,
            tc.tile_pool(name="anp", bufs=3) as anp,
            tc.tile_pool(name="attp", bufs=3) as attp,
            tc.tile_pool(name="ysbp", bufs=3) as ysbp,
            tc.tile_pool(name="pA", bufs=2, space="PSUM") as pA,
            tc.tile_pool(name="pY", bufs=2, space="PSUM") as pY,
            tc.tile_pool(name="pS", bufs=2, space="PSUM") as pS,
            tc.tile_pool(name="pG", bufs=2, space="PSUM") as pG,
        ):
            # ---- constants ----
            zbias = const.tile([P, 1], dt.float32, tag="zbias", name="zbias")
            nc.vector.memset(zbias, 0.0)
            eps_t = const.tile([1, 1], dt.float32, tag="eps", name="eps_t")
            nc.vector.memset(eps_t, 1e-5)
            ones = const.tile([P, 1], dt.bfloat16, tag="ones", name="ones")
            nc.vector.memset(ones, 1.0)
            identb = const.tile([P, P], dt.bfloat16, tag="identb", name="identb")
            make_identity(nc, identb)
            # prefetch both activation tables (Ln, Exp) with dummy ops
            dmy = const.tile([1, 2], dt.float32, tag="dmy", name="dmy")
            nc.scalar.activation(dmy[:, 0:1], eps_t,
                                 mybir.ActivationFunctionType.Ln,
                                 bias=eps_t)
            nc.scalar.activation(dmy[:, 1:2], eps_t,
                                 mybir.ActivationFunctionType.Exp,
                                 bias=zbias[0:1, :])

            # ---- DMA issue (sync queue; y outputs go on the scalar queue) ----
            wq_sb = [res.tile([P, KPAD], dt.bfloat16, tag=f"wq{n}", name=f"wq{n}")
                     for n in range(NT)]
            xt_sb = res.tile([P, KT, T], dt.bfloat16, tag="xt", name="xt")
            wout_sb = [res.tile([P, HIDDEN], dt.bfloat16, tag=f"wout{kk}",
                                name=f"wout{kk}") for kk in range(4)]
            cos_sb = const.tile([P, T], dt.float32, tag="cos", name="cos_sb")
            sin_sb = const.tile([P, T], dt.float32, tag="sin", name="sin_sb")
            mask_sb = const.tile([P, 2 * P], dt.bfloat16, tag="mask", name="mask_sb")
            esink_sb = const.tile([P, HPC], dt.float32, tag="esink", name="esink_sb")
            bqkv_sb = const.tile([P, NT], dt.float32, tag="bqkv", name="bqkv_sb")

            HK = KPAD // 2
            HO = HIDDEN // 2
            HT = T // 2

            def dma(out, in_):
                nc.sync.dma_start(out=out, in_=in_)

            def dma2(out, in_):
                nc.scalar.dma_start(out=out, in_=in_)

            # xt first (rsq path is the long pole); first tile split for an
            # early pipeline start; weights interleaved in need order
            dma(xt_sb[:, 0, 0:512], xt_d[0:P, 0:512])
            dma2(xt_sb[:, 0, 512:], xt_d[0:P, 512:])
            dma(wq_sb[4][:, :HK], wq_d[4, :, :HK])
            for ki in range(1, 5):
                (dma if ki % 2 else dma2)(xt_sb[:, ki, :],
                                          xt_d[ki * P:(ki + 1) * P, :])
            dma(wq_sb[4][:, HK:], wq_d[4, :, HK:])
            for ki in range(5, 9):
                (dma if ki % 2 else dma2)(xt_sb[:, ki, :],
                                          xt_d[ki * P:(ki + 1) * P, :])
            dma(wq_sb[0][:, :HK], wq_d[0, :, :HK])
            for ki in range(9, 13):
                (dma if ki % 2 else dma2)(xt_sb[:, ki, :],
                                          xt_d[ki * P:(ki + 1) * P, :])
            dma(wq_sb[0][:, HK:], wq_d[0, :, HK:])
            for ki in range(13, 18):
                (dma if ki % 2 else dma2)(xt_sb[:, ki, :],
                                          xt_d[ki * P:(ki + 1) * P, :])
            dma(cos_sb[:, :HT], cos_d[:, :HT])
            dma2(sin_sb[:, :HT], sin_d[:, :HT])
            for ki in range(18, KT):
                (dma if ki % 2 else dma2)(xt_sb[:, ki, :],
                                          xt_d[ki * P:(ki + 1) * P, :])
            dma(cos_sb[:, HT:], cos_d[:, HT:])
            dma2(sin_sb[:, HT:], sin_d[:, HT:])
            dma(mask_sb, mask_d)
            dma(esink_sb, esink_d)
            dma(bqkv_sb, bqkv_d)
            dma(wq_sb[1][:, :HK], wq_d[1, :, :HK])
            dma(wq_sb[1][:, HK:], wq_d[1, :, HK:])
            dma(wout_sb[0][:, :HO], wout_d[0, :, :HO])
            dma(wout_sb[0][:, HO:], wout_d[0, :, HO:])
            dma(wq_sb[2][:, :HK], wq_d[2, :, :HK])
            dma(wq_sb[2][:, HK:], wq_d[2, :, HK:])
            for kk in range(1, 4):
                dma(wout_sb[kk][:, :HO], wout_d[kk, :, :HO])
                dma(wout_sb[kk][:, HO:], wout_d[kk, :, HO:])
            dma(wq_sb[3][:, :HK], wq_d[3, :, :HK])
            dma(wq_sb[3][:, HK:], wq_d[3, :, HK:])

            # ---- sum of squares: vector bf16 squares + PE ones-reduction ----
            psum_ssq = [pY.tile([1, 512], dt.float32, tag="py", name=f"ssq{h}")
                        for h in range(2)]  # holds 2 of pY's 3 bufs until rsq
            for ki in range(KT):
                xsq = xsqp.tile([P, T], dt.bfloat16, tag="xsq", name="xsq")
                nc.vector.tensor_mul(xsq, xt_sb[:, ki, :], xt_sb[:, ki, :])
                for half in range(2):
                    nc.tensor.matmul(
                        psum_ssq[half],
                        ones,
                        xsq[:, half * 512:half * 512 + 512],
                        start=(ki == 0), stop=(ki == KT - 1),
                    )

            # rsq_b = exp(-0.5*ln(ssq/H + eps)) broadcast to 128 partitions
            lnm = res.tile([1, T], dt.float32, tag="lnm", name="lnm")
            for half in range(2):
                nc.scalar.activation(lnm[:, half * 512:half * 512 + 512],
                                     psum_ssq[half],
                                     mybir.ActivationFunctionType.Ln,
                                     bias=eps_t, scale=1.0 / HIDDEN)
            rinv = res.tile([1, T], dt.float32, tag="rinv", name="rinv")
            nc.scalar.activation(rinv, lnm, mybir.ActivationFunctionType.Exp,
                                 bias=zbias[0:1, :], scale=-0.5)
            rsq_b = res.tile([P, T], dt.float32, tag="rsq", name="rsq_b")
            nc.gpsimd.partition_broadcast(rsq_b, rinv)

            # ---- qkv projection (scaled+biased, bf16) ----
            def qkv_tile(n, half, dst):
                """dst <- (W x)*rsq + b for columns [half*512, ...+512)."""
                c0 = half * 512
                pq = pA.tile([P, 512], dt.float32, tag="pb", name="pb")
                for ki in range(KT):
                    nc.tensor.matmul(
                        pq,
                        wq_sb[n][:, ki * P:ki * P + P],
                        xt_sb[:, ki, c0:c0 + 512],
                        start=(ki == 0), stop=(ki == KT - 1),
                    )
                nc.vector.tensor_mul(dst, pq, rsq_b[:, c0:c0 + 512])
                nc.vector.tensor_scalar_add(dst, dst, bqkv_sb[:, n:n + 1])

            qra = res.tile([64, HPC, T], dt.bfloat16, tag="qra", name="qra")
            krope = res.tile([64, T], dt.bfloat16, tag="krope", name="krope")
            # all 8 token-major v tiles in one buffer; ones column prefilled
            vtok = res.tile([P, MT, AW], dt.bfloat16, tag="vtok", name="vtok")
            nc.vector.memset(vtok[:, :, HD:HD + 1], 1.0)

            qkvT4 = res.tile([P, T], dt.bfloat16, tag="qkvT4", name="qkvT4")

            def kv_phase(half):
                c0 = half * 512
                qkv_tile(4, half, qkvT4[:, c0:c0 + 512])
                for j in range(4):
                    b = half * 4 + j
                    pv = pG.tile([P, HD], dt.bfloat16, tag="pg", name="pv")
                    nc.tensor.transpose(pv, qkvT4[0:64, b * P:(b + 1) * P],
                                        identb[:64, :64])
                    nc.vector.tensor_copy(vtok[:, b, 0:HD], pv)
                # k rope (rows 64:128) at half width
                kswp = ropep.tile([P, 512], dt.bfloat16, tag="hswp", name="kswp")
                for a in (64, 96):
                    nc.scalar.copy(kswp[a:a + 32, :],
                                   qkvT4[a ^ 32:(a ^ 32) + 32, c0:c0 + 512])
                ktc = ropep.tile([P, 512], dt.bfloat16, tag="htc", name="ktc")
                nc.vector.tensor_mul(ktc[64:128, :], qkvT4[64:128, c0:c0 + 512],
                                     cos_sb[64:128, c0:c0 + 512])
                kts = ropep.tile([P, 512], dt.bfloat16, tag="hts", name="kts")
                nc.vector.tensor_mul(kts[64:128, :], kswp[64:128, :],
                                     sin_sb[64:128, c0:c0 + 512])
                nc.vector.tensor_add(krope[:, c0:c0 + 512], ktc[64:128, :],
                                     kts[64:128, :])

            def q_phase(half):
                c0 = half * 512
                for n in range(4):
                    qkvt = qkvp.tile([P, 512], dt.bfloat16, tag="qkvT",
                                     name=f"qkvt{n}")
                    qkv_tile(n, half, qkvt)
                    # rope at half width on the fly (swaps on the scalar queue)
                    swp = ropep.tile([P, 512], dt.bfloat16, tag="hswp", name="hswp")
                    for a in range(0, P, 32):
                        nc.scalar.copy(swp[a:a + 32, :],
                                       qkvt[a ^ 32:(a ^ 32) + 32, :])
                    tc_ = ropep.tile([P, 512], dt.bfloat16, tag="htc", name="htc")
                    nc.vector.tensor_mul(tc_, qkvt, cos_sb[:, c0:c0 + 512])
                    ts_ = ropep.tile([P, 512], dt.bfloat16, tag="hts", name="hts")
                    nc.vector.tensor_mul(ts_, swp, sin_sb[:, c0:c0 + 512])
                    for i in range(2):
                        b0 = 64 * i
                        nc.vector.tensor_add(qra[:, 2 * n + i, c0:c0 + 512],
                                             tc_[b0:b0 + 64, :],
                                             ts_[b0:b0 + 64, :])

            # ---- attention + out-projection for one token tile ----
            def attention(b):
                pt = ptp.tile([P, 2, HPC, P], dt.bfloat16, tag="pt", name="pt")
                kts = [(0, b - 1), (1, b)] if b > 0 else [(1, b)]
                for s, kt in kts:
                    for g in range(2):
                        ps = pS.tile([P, 4, P], dt.float32, tag="sc", name="sc")
                        nc.tensor.matmul(
                            ps,
                            krope[:, kt * P:(kt + 1) * P],
                            qra[:, 4 * g:4 * g + 4, b * P:(b + 1) * P],
                            start=True, stop=True)
                        nc.scalar.activation(pt[:, s, 4 * g:4 * g + 4, :], ps,
                                             mybir.ActivationFunctionType.Exp,
                                             bias=zbias, scale=SM_SCALE)
                    m0 = 0 if s == 0 else P
                    nc.vector.tensor_mul(pt[:, s], pt[:, s],
                                         bcast_mid(mask_sb[:, m0:m0 + P], HPC))

                # AV with ones column -> denominators in column 64
                rec8 = smallp.tile([P, HPC], dt.float32, tag="rec8", name="rec8")
                an = anp.tile([P, HPC, HD], dt.bfloat16, tag="an", name="an")
                for g in range(2):
                    pg = pG.tile([P, 4, AW], dt.float32, tag="pg", name="pg")
                    for j in range(4):
                        h = 4 * g + j
                        for idx, (s, kt) in enumerate(kts):
                            nc.tensor.matmul(pg[:, j, :], pt[:, s, h, :],
                                             vtok[:, kt, :],
                                             start=(idx == 0),
                                             stop=(idx == len(kts) - 1))
                    g0 = 4 * g
                    nc.vector.tensor_add(rec8[:, g0:g0 + 4],
                                         pg[:, :, HD:HD + 1],
                                         esink_sb[:, g0:g0 + 4])
                    nc.vector.reciprocal(rec8[:, g0:g0 + 4], rec8[:, g0:g0 + 4])
                    rec3 = bass.AP(tensor=rec8.tensor,
                                   offset=rec8[:, g0:g0 + 4].offset,
                                   ap=[rec8.ap[0], [1, 4], [0, HD]])
                    nc.vector.tensor_mul(an[:, g0:g0 + 4, :], pg[:, :, 0:HD],
                                         rec3)

                # transpose to out-proj lhsT layout [128 hd, 128 tok]
                att = attp.tile([P, 4, P], dt.bfloat16, tag="att", name="att")
                a2 = an.rearrange("p a b -> p (a b)")
                for kk in range(4):
                    pat = pG.tile([P, P], dt.bfloat16, tag="pg", name="pat")
                    nc.tensor.transpose(pat, a2[:, kk * P:(kk + 1) * P], identb)
                    if kk % 2 == 0:
                        nc.vector.tensor_copy(att[:, kk, :], pat)
                    else:
                        nc.scalar.copy(att[:, kk, :], pat)

                # out projection, bf16 out; y DMAs ride the scalar queue
                # (the last tile is drained in 480-chunks on both queues to
                # shorten the kernel tail)
                for hy in range(3):
                    ysb = ysbp.tile([P, 2, YC], dt.bfloat16, tag="ysb", name="ysb")
                    for j in range(2):
                        ch = 2 * hy + j
                        o0 = ch * YC
                        pyt = pY.tile([P, 512], dt.float32, tag="py", name="py")
                        for kk in range(4):
                            nc.tensor.matmul(pyt[:, 0:YC], att[:, kk, :],
                                             wout_sb[kk][:, o0:o0 + YC],
                                             start=(kk == 0), stop=(kk == 3))
                        if ch % 2 == 0:
                            nc.vector.tensor_copy(ysb[:, j, :], pyt[:, 0:YC])
                        else:
                            nc.scalar.copy(ysb[:, j, :], pyt[:, 0:YC])
                        if b == MT - 1:
                            for q in range(2):
                                eng = nc.sync if q == 0 else nc.scalar
                                q0 = o0 + q * (YC // 2)
                                eng.dma_start(
                                    out=y_d[b * P:(b + 1) * P, q0:q0 + YC // 2],
                                    in_=ysb[:, j, q * (YC // 2):(q + 1) * (YC // 2)])
                    if b < MT - 1:
                        nc.scalar.dma_start(
                            out=y_d[b * P:(b + 1) * P,
                                    hy * 2 * YC:(hy + 1) * 2 * YC],
                            in_=ysb)

            for half in range(2):
                kv_phase(half)
                q_phase(half)
                for j in range(4):
                    attention(half * 4 + j)

    nc.compile()
    return nc


# ----------------------------------------------------------------------------
# public entry
# ----------------------------------------------------------------------------

LAST_RESULTS = None


def kernel(x, norm_scale, qkv_w, qkv_b, out_w, out_b, sinks):
    global LAST_RESULTS
    x = np.asarray(x, dtype=np.float32)
    norm_scale = np.asarray(norm_scale, dtype=np.float32)
    qkv_w = np.asarray(qkv_w, dtype=np.float32)
    qkv_b = np.asarray(qkv_b, dtype=np.float32)
    out_w = np.asarray(out_w, dtype=np.float32)
    out_b = np.asarray(out_b, dtype=np.float32)
    sinks = np.asarray(sinks, dtype=np.float32)

    if "nc" not in _CACHE:
        _CACHE["nc"] = build_nc()
    nc = _CACHE["nc"]

    in_maps = [
        _prep_core_inputs(c, x, norm_scale, qkv_w, qkv_b, out_w, sinks)
        for c in range(NCORES)
    ]
    import os
    tmpdir = os.environ.get("BASS_TMPDIR") or None
    res = run_bass_kernel_spmd(nc, in_maps, core_ids=list(range(NCORES)),
                               tmpdir=tmpdir)
    LAST_RESULTS = res
    y = np.zeros((T, HIDDEN), dtype=np.float64)
    for c in range(NCORES):
        y += res.results[c]["y"].astype(np.float64)
    y += out_b.astype(np.float64)[None, :]
    return y.astype(np.float32)


# revision 40
# speedup vs baseline: 1.1559x; 1.0242x over previous
"""Trainium2 Bass kernel: sparse (sliding-window) attention block, v3.

Full module per reference:
  RMSNorm -> fused QKV (5120x2880) -> YaRN RoPE -> GQA sliding-window(128)
  causal attention with learned sink logit -> out projection (2880x4096).

Sharding: tensor-parallel over heads across 8 cores. Core c owns q-heads
[8c, 8c+8) and kv-head c. Each core emits a partial [1024, 2880] bf16
output; the host sums the partials (f64) and adds out_b.

v3 = v1 attention backend (q-on-partitions AV with ones-column
denominators, tiny reciprocals, PE transposes for the out-proj lhsT)
plus the v2 scheduling wins:
  - both activation tables (Ln, Exp) prefetched with dummy ops at t0; no
    Square activation anywhere (x^2 via vector bf16 muls) -> no table
    swaps, no startup stall.
  - half-granularity pipeline: kv+q projection and rope for tokens
    [0,512) are followed immediately by attention+out-proj of tiles 0-3
    while the second half's QKV matmuls still run.
  - sm_scale folded into the Exp activation scale -> q and k share one
    plain cos/sin table pair (half the table DMA/SBUF).
  - masks multiplicative {0,1} bf16 applied to the exp'd probabilities.
  - y written bf16 in [128, 960] chunks DMA'd from the scalar queue.
"""

import math
import sys

import numpy as np

try:
    import concourse.bass as bass
except ImportError:  # pragma: no cover
    sys.path.insert(0, "/opt/trn_rl_repo")
    import concourse.bass as bass

import concourse.bacc as bacc
import concourse.tile as tile
from concourse import mybir
from concourse.masks import make_identity
from concourse.bass_utils import run_bass_kernel_spmd

import ml_dtypes

BF16 = ml_dtypes.bfloat16

T = 1024
HIDDEN = 2880
HD = 64
NH = 64
NKV = 8
SW = 128
NCORES = 8
HPC = NH // NCORES          # q heads per core = 8
QKV_DIM = HD * (NH + 2 * NKV)
SM_SCALE = 1.0 / math.sqrt(HD)

P = 128
KT = (HIDDEN + P - 1) // P   # 23 k-tiles over hidden (zero-padded to 2944)
KPAD = KT * P
NT = 5                       # qkv n-tiles of 128 (4 q-tiles + 1 kv-tile)
MT = T // P                  # 8 token tiles
AW = HD + 1                  # AV width: 64 v dims + denominator column

dt = mybir.dt

_CACHE = {}


# ----------------------------------------------------------------------------
# host-side helpers
# ----------------------------------------------------------------------------

def _rope_cos_sin(num_tokens):
    base = 150000.0
    scaling = 32.0
    init_ctx = 4096.0
    ntk_alpha = 1.0
    ntk_beta = 32.0
    d_half = HD / 2
    freq = base ** (np.arange(0, HD, 2, dtype=np.float32) / HD)
    concentration = 0.1 * math.log(scaling) + 1.0
    low = d_half * math.log(init_ctx / (ntk_beta * 2 * math.pi)) / math.log(base)
    high = d_half * math.log(init_ctx / (ntk_alpha * 2 * math.pi)) / math.log(base)
    interpolation = 1.0 / (scaling * freq)
    extrapolation = 1.0 / freq
    ramp = (np.arange(int(d_half), dtype=np.float32) - low) / (high - low)
    m = 1.0 - np.clip(ramp, 0.0, 1.0)
    inv_freq = interpolation * (1.0 - m) + extrapolation * m
    t = np.arange(num_tokens, dtype=np.float32)
    freqs = t[:, None] * inv_freq[None, :]
    cos = (np.cos(freqs) * concentration).astype(np.float32)
    sin = (np.sin(freqs) * concentration).astype(np.float32)
    return cos, sin  # [T, 32]


def _host_tables():
    """Plain (unscaled) replicated rope tables with the swap sign folded
    into sin: rope(u)[p] = u[p]*cos[p] + u[p^32]*sin_alt[p]."""
    cos, sin = _rope_cos_sin(T)  # [1024, 32]
    sgn = np.repeat([-1.0, 1.0], 32)[:, None].astype(np.float32)
    sgn = np.tile(sgn, (2, 1))  # [128, 1]
    cos_t = np.tile(cos.T, (4, 1)).astype(np.float32)          # [128, 1024]
    sin_t = (np.tile(sin.T, (4, 1)) * sgn).astype(np.float32)  # [128, 1024]
    return cos_t, sin_t


def _host_masks01():
    j = np.arange(P)[:, None]   # kt row (partition)
    i = np.arange(P)[None, :]   # q col (free)
    mask_prev = (j > i).astype(np.float32)    # dist in [1,127]
    mask_self = (j <= i).astype(np.float32)   # dist in [0,127]
    return np.concatenate([mask_prev, mask_self], axis=1).astype(BF16)


def _prep_core_inputs(core, x, norm_scale, qkv_w, qkv_b, out_w, sinks):
    q_end = NH * HD
    k_end = q_end + NKV * HD

    qrows = np.arange(core * HPC * HD, (core + 1) * HPC * HD)
    krows = np.arange(q_end + core * HD, q_end + (core + 1) * HD)
    vrows = np.arange(k_end + core * HD, k_end + (core + 1) * HD)
    # kv n-tile: v in partitions 0:64, k in 64:128
    rows = np.concatenate([qrows, vrows, krows])  # [640]

    wshard = (qkv_w[rows, :] * norm_scale[None, :]).astype(np.float32)
    bshard = qkv_b[rows].astype(np.float32)  # [640]

    # lhsT tiles: wq[n, kp, kt*128 + nc] = wshard[n*128 + nc, kt*128 + kp]
    wq = np.zeros((NT, P, KPAD), dtype=BF16)
    for n in range(NT):
        blk = wshard[n * P:(n + 1) * P, :]  # [128 n, 2880 k]
        for ki in range(KT):
            k0 = ki * P
            ksz = min(P, HIDDEN - k0)
            wq[n, :ksz, ki * P:ki * P + P] = blk[:, k0:k0 + ksz].T.astype(BF16)

    cols = np.arange(core * HPC * HD, (core + 1) * HPC * HD)
    wo = out_w[:, cols].T.astype(np.float32)  # [512 hd, 2880 H]
    wout = wo.reshape(4, P, HIDDEN).astype(BF16)

    bqkv = bshard.reshape(NT, P).T.copy().astype(np.float32)  # [128, 5]

    cos_t, sin_t = _host_tables()  # [128, 1024] f32 each

    xt = np.zeros((KPAD, T), dtype=BF16)
    xt[:HIDDEN] = x.T.astype(BF16)

    esink = np.exp(sinks[core * HPC:(core + 1) * HPC].astype(np.float64))
    esink = np.broadcast_to(esink.astype(np.float32), (P, HPC)).copy()

    return {
        "xt": xt,                               # [2944, 1024] bf16
        "wq": wq,                               # [5, 128, 2944] bf16
        "wout": wout,                           # [4, 128, 2880] bf16
        "bqkv": bqkv,                           # [128, 5] f32
        "cos_t": cos_t, "sin_t": sin_t,         # [128, 1024] f32
        "sin_neg": (-sin_t).copy(),             # sin_alt[p^32] = -sin_alt[p]
        "mask": _host_masks01(),                # [128, 256] bf16
        "esink": esink,                         # [128, 8] f32
    }


# ----------------------------------------------------------------------------
# device kernel (Tile)
# ----------------------------------------------------------------------------

def build_nc():
    nc = bacc.Bacc("TRN2", target_bir_lowering=False, debug=False)

    xt_d = nc.dram_tensor("xt", [KPAD, T], dt.bfloat16, kind="ExternalInput").ap()
    wq_d = nc.dram_tensor("wq", [NT, P, KPAD], dt.bfloat16, kind="ExternalInput").ap()
    wout_d = nc.dram_tensor("wout", [4, P, HIDDEN], dt.bfloat16, kind="ExternalInput").ap()
    bqkv_d = nc.dram_tensor("bqkv", [P, NT], dt.float32, kind="ExternalInput").ap()
    cos_d = nc.dram_tensor("cos_t", [P, T], dt.float32, kind="ExternalInput").ap()
    sin_d = nc.dram_tensor("sin_t", [P, T], dt.float32, kind="ExternalInput").ap()
    sneg_d = nc.dram_tensor("sin_neg", [P, T], dt.float32, kind="ExternalInput").ap()
    mask_d = nc.dram_tensor("mask", [P, 2 * P], dt.bfloat16, kind="ExternalInput").ap()
    esink_d = nc.dram_tensor("esink", [P, HPC], dt.float32, kind="ExternalInput").ap()
    y_d = nc.dram_tensor("y", [T, HIDDEN], dt.bfloat16, kind="ExternalOutput").ap()

    YC = 480

    def bcast_mid(ap2d, n):
        """[P, F] -> [P, n, F] with a 0-step middle dim (free broadcast)."""
        return bass.AP(tensor=ap2d.tensor, offset=ap2d.offset,
                       ap=[ap2d.ap[0], [0, n]] + list(ap2d.ap[1:]))

    with tile.TileContext(nc) as tc:
        with (
            tc.tile_pool(name="const", bufs=1) as const,
            tc.tile_pool(name="res", bufs=1) as res,
            tc.tile_pool(name="qkvp", bufs=2) as qkvp,
            tc.tile_pool(name="xsqp", bufs=2) as xsqp,
            tc.tile_pool(name="ropep", bufs=2) as ropep,
            tc.tile_pool(name="ptp", bufs=3) as ptp,
            tc.tile_pool(name="smallp", bufs=3) as smallp,
            tc.tile_pool(name="anp", bufs=3) as anp,
            tc.tile_pool(name="attp", bufs=3) as attp,
            tc.tile_pool(name="ysbp", bufs=3) as ysbp,
            tc.tile_pool(name="pA", bufs=2, space="PSUM") as pA,
            tc.tile_pool(name="pY", bufs=2, space="PSUM") as pY,
            tc.tile_pool(name="pS", bufs=2, space="PSUM") as pS,
            tc.tile_pool(name="pG", bufs=2, space="PSUM") as pG,
        ):
            # ---- constants ----
            zbias = const.tile([P, 1], dt.float32, tag="zbias", name="zbias")
            nc.vector.memset(zbias, 0.0)
            eps_t = const.tile([1, 1], dt.float32, tag="eps", name="eps_t")
            nc.vector.memset(eps_t, 1e-5)
            ones = const.tile([P, 1], dt.bfloat16, tag="ones", name="ones")
            nc.vector.memset(ones, 1.0)
            identb = const.tile([P, P], dt.bfloat16, tag="identb", name="identb")
            make_identity(nc, identb)
            # prefetch both activation tables (Ln, Exp) with dummy ops
            dmy = const.tile([1, 2], dt.float32, tag="dmy", name="dmy")
            nc.scalar.activation(dmy[:, 0:1], eps_t,
                                 mybir.ActivationFunctionType.Ln,
                                 bias=eps_t)
            nc.scalar.activation(dmy[:, 1:2], eps_t,
                                 mybir.ActivationFunctionType.Exp,
                                 bias=zbias[0:1, :])

            # ---- DMA issue (sync queue; y outputs go on the scalar queue) ----
            wq_sb = [res.tile([P, KPAD], dt.bfloat16, tag=f"wq{n}", name=f"wq{n}")
                     for n in range(NT)]
            xt_sb = res.tile([P, KT, T], dt.bfloat16, tag="xt", name="xt")
            wout_sb = [res.tile([P, HIDDEN], dt.bfloat16, tag=f"wout{kk}",
                                name=f"wout{kk}") for kk in range(4)]
            cos_sb = const.tile([P, T], dt.float32, tag="cos", name="cos_sb")
            sin_sb = const.tile([P, T], dt.float32, tag="sin", name="sin_sb")
            sneg_sb = const.tile([P, T], dt.float32, tag="sneg", name="sneg_sb")
            mask_sb = const.tile([P, 2 * P], dt.bfloat16, tag="mask", name="mask_sb")
            esink_sb = const.tile([P, HPC], dt.float32, tag="esink", name="esink_sb")
            bqkv_sb = const.tile([P, NT], dt.float32, tag="bqkv", name="bqkv_sb")

            HK = KPAD // 2
            HO = HIDDEN // 2
            HT = T // 2

            def dma(out, in_):
                nc.sync.dma_start(out=out, in_=in_)

            def dma2(out, in_):
                nc.scalar.dma_start(out=out, in_=in_)

            # xt first (rsq path is the long pole); first tile split for an
            # early pipeline start; weights interleaved in need order
            dma(xt_sb[:, 0, 0:512], xt_d[0:P, 0:512])
            dma2(xt_sb[:, 0, 512:], xt_d[0:P, 512:])
            dma(wq_sb[4][:, :HK], wq_d[4, :, :HK])
            for ki in range(1, 5):
                (dma if ki % 2 else dma2)(xt_sb[:, ki, :],
                                          xt_d[ki * P:(ki + 1) * P, :])
            dma(wq_sb[4][:, HK:], wq_d[4, :, HK:])
            for ki in range(5, 9):
                (dma if ki % 2 else dma2)(xt_sb[:, ki, :],
                                          xt_d[ki * P:(ki + 1) * P, :])
            dma(wq_sb[0][:, :HK], wq_d[0, :, :HK])
            for ki in range(9, 13):
                (dma if ki % 2 else dma2)(xt_sb[:, ki, :],
                                          xt_d[ki * P:(ki + 1) * P, :])
            dma(wq_sb[0][:, HK:], wq_d[0, :, HK:])
            for ki in range(13, 18):
                (dma if ki % 2 else dma2)(xt_sb[:, ki, :],
                                          xt_d[ki * P:(ki + 1) * P, :])
            dma(cos_sb[:, :HT], cos_d[:, :HT])
            dma2(sin_sb[:, :HT], sin_d[:, :HT])
            dma(sneg_sb[:, :HT], sneg_d[:, :HT])
            for ki in range(18, KT):
                (dma if ki % 2 else dma2)(xt_sb[:, ki, :],
                                          xt_d[ki * P:(ki + 1) * P, :])
            dma(cos_sb[:, HT:], cos_d[:, HT:])
            dma2(sin_sb[:, HT:], sin_d[:, HT:])
            dma2(sneg_sb[:, HT:], sneg_d[:, HT:])
            dma(mask_sb, mask_d)
            dma(esink_sb, esink_d)
            dma(bqkv_sb, bqkv_d)
            dma(wq_sb[1][:, :HK], wq_d[1, :, :HK])
            dma(wq_sb[1][:, HK:], wq_d[1, :, HK:])
            dma(wout_sb[0][:, :HO], wout_d[0, :, :HO])
            dma(wout_sb[0][:, HO:], wout_d[0, :, HO:])
            dma(wq_sb[2][:, :HK], wq_d[2, :, :HK])
            dma(wq_sb[2][:, HK:], wq_d[2, :, HK:])
            for kk in range(1, 4):
                dma(wout_sb[kk][:, :HO], wout_d[kk, :, :HO])
                dma(wout_sb[kk][:, HO:], wout_d[kk, :, HO:])
            dma(wq_sb[3][:, :HK], wq_d[3, :, :HK])
            dma(wq_sb[3][:, HK:], wq_d[3, :, HK:])

            # ---- sum of squares: vector bf16 squares + PE ones-reduction ----
            psum_ssq = [pY.tile([1, 512], dt.float32, tag="py", name=f"ssq{h}")
                        for h in range(2)]  # holds 2 of pY's 3 bufs until rsq
            for ki in range(KT):
                xsq = xsqp.tile([P, T], dt.bfloat16, tag="xsq", name="xsq")
                nc.vector.tensor_mul(xsq, xt_sb[:, ki, :], xt_sb[:, ki, :])
                for half in range(2):
                    nc.tensor.matmul(
                        psum_ssq[half],
                        ones,
                        xsq[:, half * 512:half * 512 + 512],
                        start=(ki == 0), stop=(ki == KT - 1),
                    )

            # rsq_b = exp(-0.5*ln(ssq/H + eps)) broadcast to 128 partitions
            lnm = res.tile([1, T], dt.float32, tag="lnm", name="lnm")
            for half in range(2):
                nc.scalar.activation(lnm[:, half * 512:half * 512 + 512],
                                     psum_ssq[half],
                                     mybir.ActivationFunctionType.Ln,
                                     bias=eps_t, scale=1.0 / HIDDEN)
            rinv = res.tile([1, T], dt.float32, tag="rinv", name="rinv")
            nc.scalar.activation(rinv, lnm, mybir.ActivationFunctionType.Exp,
                                 bias=zbias[0:1, :], scale=-0.5)
            rsq_b = res.tile([P, T], dt.float32, tag="rsq", name="rsq_b")
            nc.gpsimd.partition_broadcast(rsq_b, rinv)

            # ---- qkv projection (scaled+biased, bf16) ----
            def qkv_tile(n, half, dst):
                """dst <- (W x)*rsq + b for columns [half*512, ...+512)."""
                c0 = half * 512
                pq = pA.tile([P, 512], dt.float32, tag="pb", name="pb")
                for ki in range(KT):
                    nc.tensor.matmul(
                        pq,
                        wq_sb[n][:, ki * P:ki * P + P],
                        xt_sb[:, ki, c0:c0 + 512],
                        start=(ki == 0), stop=(ki == KT - 1),
                    )
                nc.vector.tensor_mul(dst, pq, rsq_b[:, c0:c0 + 512])
                nc.vector.tensor_scalar_add(dst, dst, bqkv_sb[:, n:n + 1])

            qra = res.tile([64, HPC, T], dt.bfloat16, tag="qra", name="qra")
            krope = res.tile([64, T], dt.bfloat16, tag="krope", name="krope")
            # all 8 token-major v tiles in one buffer; ones column prefilled
            vtok = res.tile([P, MT, AW], dt.bfloat16, tag="vtok", name="vtok")
            nc.vector.memset(vtok[:, :, HD:HD + 1], 1.0)

            qkvT4 = res.tile([P, T], dt.bfloat16, tag="qkvT4", name="qkvT4")

            def kv_phase(half):
                c0 = half * 512
                qkv_tile(4, half, qkvT4[:, c0:c0 + 512])
                for j in range(4):
                    b = half * 4 + j
                    pv = pG.tile([P, HD], dt.bfloat16, tag="pg", name="pv")
                    nc.tensor.transpose(pv, qkvT4[0:64, b * P:(b + 1) * P],
                                        identb[:64, :64])
                    nc.vector.tensor_copy(vtok[:, b, 0:HD], pv)
                # k rope (rows 64:128) at half width; the swapped-operand term
                # is computed with partition-offset muls (no copies)
                ktc = ropep.tile([P, 512], dt.bfloat16, tag="htc", name="ktc")
                nc.vector.tensor_mul(ktc[64:128, :], qkvT4[64:128, c0:c0 + 512],
                                     cos_sb[64:128, c0:c0 + 512])
                kts = ropep.tile([P, 512], dt.bfloat16, tag="hts", name="kts")
                for a in (64, 96):
                    b_ = a ^ 32
                    nc.vector.tensor_mul(kts[a:a + 32, :],
                                         qkvT4[b_:b_ + 32, c0:c0 + 512],
                                         sneg_sb[b_:b_ + 32, c0:c0 + 512])
                nc.vector.tensor_add(krope[:, c0:c0 + 512], ktc[64:128, :],
                                     kts[64:128, :])

            def q_phase(half):
                c0 = half * 512
                for n in range(4):
                    qkvt = qkvp.tile([P, 512], dt.bfloat16, tag="qkvT",
                                     name=f"qkvt{n}")
                    qkv_tile(n, half, qkvt)
                    # rope at half width; swapped term via partition-offset muls
                    tc_ = ropep.tile([P, 512], dt.bfloat16, tag="htc", name="htc")
                    nc.vector.tensor_mul(tc_, qkvt, cos_sb[:, c0:c0 + 512])
                    ts_ = ropep.tile([P, 512], dt.bfloat16, tag="hts", name="hts")
                    for a in range(0, P, 32):
                        b_ = a ^ 32
                        nc.vector.tensor_mul(ts_[a:a + 32, :],
                                             qkvt[b_:b_ + 32, :],
                                             sneg_sb[b_:b_ + 32, c0:c0 + 512])
                    for i in range(2):
                        b0 = 64 * i
                        nc.vector.tensor_add(qra[:, 2 * n + i, c0:c0 + 512],
                                             tc_[b0:b0 + 64, :],
                                             ts_[b0:b0 + 64, :])

            # ---- attention + out-projection for one token tile ----
            def attention(b):
                pt = ptp.tile([P, 2, HPC, P], dt.bfloat16, tag="pt", name="pt")
                kts = [(0, b - 1), (1, b)] if b > 0 else [(1, b)]
                # per-group pipeline: group 0 (heads 0-3, q-tiles 0/1) runs
                # fully while the later q-tiles' rope may still be in flight
                rec8 = smallp.tile([P, HPC], dt.float32, tag="rec8", name="rec8")
                an = anp.tile([P, HPC, HD], dt.bfloat16, tag="an", name="an")
                att = attp.tile([P, 4, P], dt.bfloat16, tag="att", name="att")
                a2 = an.rearrange("p a b -> p (a b)")
                for g in range(2):
                    g0 = 4 * g
                    for s, kt in kts:
                        ps = pS.tile([P, 4, P], dt.float32, tag="sc", name="sc")
                        nc.tensor.matmul(
                            ps,
                            krope[:, kt * P:(kt + 1) * P],
                            qra[:, g0:g0 + 4, b * P:(b + 1) * P],
                            start=True, stop=True)
                        nc.scalar.activation(pt[:, s, g0:g0 + 4, :], ps,
                                             mybir.ActivationFunctionType.Exp,
                                             bias=zbias, scale=SM_SCALE)
                        m0 = 0 if s == 0 else P
                        nc.vector.tensor_mul(pt[:, s, g0:g0 + 4, :],
                                             pt[:, s, g0:g0 + 4, :],
                                             bcast_mid(mask_sb[:, m0:m0 + P], 4))
                    # AV with ones column -> denominators in column 64
                    pg = pG.tile([P, 4, AW], dt.float32, tag="pg", name="pg")
                    for j in range(4):
                        h = g0 + j
                        for idx, (s, kt) in enumerate(kts):
                            nc.tensor.matmul(pg[:, j, :], pt[:, s, h, :],
                                             vtok[:, kt, :],
                                             start=(idx == 0),
                                             stop=(idx == len(kts) - 1))
                    nc.vector.tensor_add(rec8[:, g0:g0 + 4],
                                         pg[:, :, HD:HD + 1],
                                         esink_sb[:, g0:g0 + 4])
                    nc.vector.reciprocal(rec8[:, g0:g0 + 4], rec8[:, g0:g0 + 4])
                    rec3 = bass.AP(tensor=rec8.tensor,
                                   offset=rec8[:, g0:g0 + 4].offset,
                                   ap=[rec8.ap[0], [1, 4], [0, HD]])
                    nc.vector.tensor_mul(an[:, g0:g0 + 4, :], pg[:, :, 0:HD],
                                         rec3)
                    # transpose to out-proj lhsT layout [128 hd, 128 tok]
                    for jj in range(2):
                        kk = 2 * g + jj
                        pat = pG.tile([P, P], dt.bfloat16, tag="pg", name="pat")
                        nc.tensor.transpose(pat, a2[:, kk * P:(kk + 1) * P],
                                            identb)
                        if kk % 2 == 0:
                            nc.vector.tensor_copy(att[:, kk, :], pat)
                        else:
                            nc.scalar.copy(att[:, kk, :], pat)

                # out projection, bf16 out; y DMAs ride the scalar queue
                # (the last tile is drained in 480-chunks on both queues to
                # shorten the kernel tail)
                for hy in range(3):
                    ysb = ysbp.tile([P, 2, YC], dt.bfloat16, tag="ysb", name="ysb")
                    for j in range(2):
                        ch = 2 * hy + j
                        o0 = ch * YC
                        pyt = pY.tile([P, 512], dt.float32, tag="py", name="py")
                        for kk in range(4):
                            nc.tensor.matmul(pyt[:, 0:YC], att[:, kk, :],
                                             wout_sb[kk][:, o0:o0 + YC],
                                             start=(kk == 0), stop=(kk == 3))
                        if ch % 2 == 0:
                            nc.vector.tensor_copy(ysb[:, j, :], pyt[:, 0:YC])
                        else:
                            nc.scalar.copy(ysb[:, j, :], pyt[:, 0:YC])
                        if b == MT - 1:
                            for q in range(2):
                                eng = nc.sync if q == 0 else nc.scalar
                                q0 = o0 + q * (YC // 2)
                                eng.dma_start(
                                    out=y_d[b * P:(b + 1) * P, q0:q0 + YC // 2],
                                    in_=ysb[:, j, q * (YC // 2):(q + 1) * (YC // 2)])
                    if b < MT - 1:
                        nc.sync.dma_start(
                            out=y_d[b * P:(b + 1) * P,
                                    hy * 2 * YC:(hy + 1) * 2 * YC],
                            in_=ysb)

            for half in range(2):
                kv_phase(half)
                q_phase(half)
                for j in range(4):
                    attention(half * 4 + j)

    nc.compile()
    return nc


# ----------------------------------------------------------------------------
# public entry
# ----------------------------------------------------------------------------

LAST_RESULTS = None


def kernel(x, norm_scale, qkv_w, qkv_b, out_w, out_b, sinks):
    global LAST_RESULTS
    x = np.asarray(x, dtype=np.float32)
    norm_scale = np.asarray(norm_scale, dtype=np.float32)
    qkv_w = np.asarray(qkv_w, dtype=np.float32)
    qkv_b = np.asarray(qkv_b, dtype=np.float32)
    out_w = np.asarray(out_w, dtype=np.float32)
    out_b = np.asarray(out_b, dtype=np.float32)
    sinks = np.asarray(sinks, dtype=np.float32)

    if "nc" not in _CACHE:
        _CACHE["nc"] = build_nc()
    nc = _CACHE["nc"]

    in_maps = [
        _prep_core_inputs(c, x, norm_scale, qkv_w, qkv_b, out_w, sinks)
        for c in range(NCORES)
    ]
    import os
    tmpdir = os.environ.get("BASS_TMPDIR") or None
    res = run_bass_kernel_spmd(nc, in_maps, core_ids=list(range(NCORES)),
                               tmpdir=tmpdir)
    LAST_RESULTS = res
    y = np.zeros((T, HIDDEN), dtype=np.float64)
    for c in range(NCORES):
        y += res.results[c]["y"].astype(np.float64)
    y += out_b.astype(np.float64)[None, :]
    return y.astype(np.float32)
